# revision 23
# baseline (speedup 1.0000x reference)
"""Trainium2 Bass kernel for DSAttention (causal attention with per-batch tau
scale and per-key delta bias), B=4 L=S=2048 H=8 E=64 fp32.

Strategy: data+head parallelism across 8 cores (core i -> batch i//2, heads
(i%2)*4 .. +4, i.e. 4 (b,h) pairs per core). Per pair, flash-style attention
computed entirely in the transposed-score layout:

  scoresT[s, l] = sum_e K[s,e] Q[l,e]          (PE, float32r, K/Q pre-transposed
                                                on host so E sits on partitions)
  AT[s, l] = exp((tau/8)*scoresT + delta[s]/8) (ACT; tau via per-partition scale
                                                AP, delta via per-partition bias
                                                AP; no max-subtraction needed --
                                                scores are bounded ~|9|)
  OT[e', l] += V'[s, e'].T @ AT[s, l]          (PE accumulates over s-tiles in
                                                PSUM; V' has a ones column so
                                                row 64 of OT is the softmax
                                                denominator)
  out[l, e] = OT[e, l] / OT[64, l]             (PE transpose of OT chunks, then
                                                DVE reciprocal + scalar mul)

Causality: s-tile si only contributes to columns l >= 128*si; the triangular
diagonal block is masked by adding -1e30 before the exp.
"""

import sys

if "/opt/trn_rl_repo" not in sys.path:
    sys.path.insert(0, "/opt/trn_rl_repo")

import numpy as np

import concourse.bacc as bacc
import concourse.mybir as mybir
import concourse.tile as tile
from concourse import bass_utils

B, L, S, H, E = 4, 2048, 2048, 8, 64
N_CORES = 8
PAIRS = 4          # (b, h) pairs per core
NT = S // 128      # s-tiles per pair
CHUNK = 512        # scoresT psum chunk width (1 PSUM bank)
PHW = 1024         # phase width: OT accumulator columns per phase
F32 = mybir.dt.float32
F32R = mybir.dt.float32r
NEG = -1.0e30


def _pieces(lo, hi):
    """Split [lo, hi) at absolute 512 boundaries (PSUM bank alignment)."""
    out = []
    a = lo
    while a < hi:
        b = min(hi, (a // 512 + 1) * 512)
        out.append((a, b))
        a = b
    return out


def _body(tc, nc, qt_d, kt_d, vv_d, deltas_d, taub_d, mask_d, eye_d, out_d):
    Exp = mybir.ActivationFunctionType.Exp
    with (
        tc.tile_pool(name="const", bufs=1) as cp,
        tc.tile_pool(name="io", bufs=2) as iop,
        tc.tile_pool(name="at", bufs=3) as atp,
        tc.tile_pool(name="outp", bufs=2) as op,
        tc.tile_pool(name="ps_s", bufs=3, space="PSUM") as pss,
        tc.tile_pool(name="ps_ot", bufs=2, space="PSUM") as psot,
        tc.tile_pool(name="ps_tr", bufs=1, space="PSUM") as trp,
    ):
        # ---- constants (per core; the core's 4 pairs share one batch b) ----
        delta_raw = cp.tile([128, NT], F32)
        nc.sync.dma_start(delta_raw[:], deltas_d.ap())
        delta_sc = cp.tile([128, NT], F32)
        nc.vector.tensor_scalar_mul(delta_sc[:], delta_raw[:], 0.125)
        tau_raw = cp.tile([128, 1], F32)
        nc.sync.dma_start(tau_raw[:], taub_d.ap())
        tau_sc = cp.tile([128, 1], F32)
        nc.vector.tensor_scalar_mul(tau_sc[:], tau_raw[:], 0.125)
        mask_t = cp.tile([128, 128], F32R)
        nc.sync.dma_start(mask_t[:], mask_d.ap())
        eye_t = cp.tile([65, 65], F32)
        nc.sync.dma_start(eye_t[:], eye_d.ap())

        # ---- software-pipelined emission over all (pair, si, chunk) units ----
        # PE program order interleaves mm1(u+1) ahead of mm2(u) so the PE can
        # compute the next scores block while ACT runs the exp of the current
        # one; per-pair tail work (PSUM drain, transpose, normalize, store) is
        # spread across the following pair's units.
        def load_pair(p):
            qt_t = iop.tile([E, L], F32R, tag="qt")
            kt_t = iop.tile([E, S], F32R, tag="kt")
            vp_t = iop.tile([128, NT * 65], F32R, tag="vp")
            vp3 = vp_t[:].rearrange("p (s c) -> p s c", c=65)
            vsrc = vv_d.ap()[p].rearrange("(s p) e -> p s e", p=128)
            # head pieces first so the first units' matmuls start ASAP
            nc.sync.dma_start(qt_t[:, 0:512], qt_d.ap()[p][:, 0:512])
            nc.sync.dma_start(kt_t[:, 0:512], kt_d.ap()[p][:, 0:512])
            nc.sync.dma_start(vp3[:, 0:4, :], vsrc[:, 0:4, :])
            nc.sync.dma_start(qt_t[:, 512:L], qt_d.ap()[p][:, 512:L])
            nc.sync.dma_start(kt_t[:, 512:S], kt_d.ap()[p][:, 512:S])
            nc.sync.dma_start(vp3[:, 4:NT, :], vsrc[:, 4:NT, :])
            # fold tau/8 into Q (avoids the activation scale-as-AP path);
            # piecewise so the first scores matmul only waits on 512 columns
            for z in range(0, L, 512):
                nc.vector.tensor_scalar_mul(
                    qt_t[:, z : z + 512], qt_t[:, z : z + 512], tau_sc[0:E, 0:1]
                )
            return qt_t, kt_t, vp3

        # units: (pair, phase, si, chunk); each phase covers PHW output
        # columns so its OT accumulator is only 2 PSUM banks (double-buffered),
        # which frees a dedicated bank for the tail transposes.
        units = []
        for p in range(PAIRS):
            for ph in range(L // PHW):
                for si in range(NT):
                    for c in range((PHW // CHUNK) * ph, (PHW // CHUNK) * (ph + 1)):
                        if c >= si * 128 // CHUNK:
                            units.append((p, ph, si, c))

        tiles = {0: load_pair(0)}
        ots = {}
        drains = {}  # pair -> (ot_sb, o_all)
        tail = []  # queued closures, a few popped per unit
        pending = []  # (unit, at, lo) with mm2 not yet emitted

        def emit_mm2(u, at, lo):
            p, ph, si, c = u
            _, _, vp3 = tiles[p]
            for a, b in _pieces(lo, (c + 1) * CHUNK):
                bank = a // 512
                nc.tensor.matmul(
                    ots[(p, ph)][:, a - PHW * ph : b - PHW * ph],
                    vp3[:, si, :],
                    at[:, a - lo : b - lo],
                    start=(si == 0),
                    stop=(si == 4 * bank + 3),
                )
            # OT bank `bk` is final once s-tile 4*bk+3 is accumulated; drain
            # it to SBUF immediately and queue its normalize/store tail steps
            # so tail work overlaps the remaining units.
            for a, b in _pieces(lo, (c + 1) * CHUNK):
                bk = a // 512
                if si == 4 * bk + 3:
                    if p not in drains:
                        ot_sb = op.tile([65, L], F32, tag="otsb", name=f"otsb{p}")
                        o_all = op.tile(
                            [128, (L // 128) * 64], F32, tag="oall", name=f"oall{p}"
                        )
                        drains[p] = (ot_sb, o_all)
                    ot_sb = drains[p][0]
                    nc.vector.tensor_copy(
                        ot_sb[:, 512 * bk : 512 * (bk + 1)],
                        ots[(p, ph)][
                            :, 512 * bk - PHW * ph : 512 * (bk + 1) - PHW * ph
                        ],
                    )
                    queue_bank_tail(p, bk)

        def queue_bank_tail(p, bk):
            ot_sb, o_all = drains[p]

            def step(t):
                # alternate the transpose landing bank between the dedicated
                # pool and a spare scores slot so consecutive steps overlap
                if t % 2 == 0:
                    tr = trp.tile([128, 65], F32, tag="tr")
                else:
                    tr = pss.tile([128, 65], F32, tag="st")
                nc.tensor.transpose(tr[:], ot_sb[:, 128 * t : 128 * (t + 1)], eye_t[:])
                r = op.tile([128, 1], F32, tag="r")
                nc.vector.reciprocal(r[:], tr[:, 64:65])
                nc.vector.tensor_scalar_mul(
                    o_all[:, 64 * t : 64 * (t + 1)], tr[:, 0:64], r[:, 0:1]
                )
                if t % 4 == 3:  # one store per drained OT bank (4 l-tiles)
                    t0 = t - 3
                    nc.sync.dma_start(
                        out_d.ap()[p, 128 * t0 : 128 * (t + 1), :].rearrange(
                            "(t pp) e -> pp t e", pp=128
                        ),
                        o_all[:, 64 * t0 : 64 * (t + 1)].rearrange(
                            "pp (t e) -> pp t e", e=64
                        ),
                    )

            for t in range(4 * bk, 4 * (bk + 1)):
                tail.append((step, t))

        # index of the unit at which to prefetch the next pair's inputs
        first_unit = {}
        for i, uu in enumerate(units):
            first_unit.setdefault(uu[0], i)
        prefetch_at = {
            first_unit[p + 1] - 8: p + 1 for p in range(PAIRS - 1)
        }

        for i, u in enumerate(units):
            p, ph, si, c = u
            if i in prefetch_at:
                q = prefetch_at[i]
                tiles[q] = load_pair(q)
            if (p, ph) not in ots:
                ots[(p, ph)] = psot.tile(
                    [65, PHW], F32, tag="ot", name=f"ot{p}_{ph}"
                )
            qt_t, kt_t, _ = tiles[p]
            cl = c * CHUNK
            cr = cl + CHUNK
            l0 = si * 128
            lo = max(l0, cl)
            st = pss.tile([128, CHUNK], F32, tag="st")
            for a, b in _pieces(lo, cr):
                nc.tensor.matmul(
                    st[:, a - cl : b - cl],
                    kt_t[:, l0 : l0 + 128],
                    qt_t[:, a:b],
                    start=True,
                    stop=True,
                )
            # tail work (PE transposes + DVE norm) emitted here keeps the DVE
            # FIFO ahead of the post-exp mask below.
            for _ in range(2):
                if tail:
                    fn, t = tail.pop(0)
                    fn(t)
            at = atp.tile([128, CHUNK], F32R, tag="at")
            nc.scalar.activation(
                at[:, 0 : cr - lo],
                st[:, lo - cl : CHUNK],
                Exp,
                bias=delta_sc[:, si : si + 1],
                scale=1.0,
            )
            if cl <= l0 < cr:
                # causal mask: zero the strict lower triangle of the diagonal
                # block AFTER the exp (cheaper than gating the exp on a DVE
                # mask-add; exp inputs are bounded so no overflow risk)
                od = l0 - lo
                nc.vector.tensor_mul(
                    at[:, od : od + 128], at[:, od : od + 128], mask_t[:]
                )
            while len(pending) >= 1:
                q = pending.pop(0)
                emit_mm2(*q)
            pending.append((u, at, lo))

        for q in pending:
            emit_mm2(*q)
        while tail:
            fn, t = tail.pop(0)
            fn(t)


_CACHE = {}


def _build():
    if "nc" in _CACHE:
        return _CACHE["nc"]
    nc = bacc.Bacc("TRN2", target_bir_lowering=False, debug=False, num_devices=N_CORES)
    qt_d = nc.dram_tensor("qt", [PAIRS, E, L], F32R, kind="ExternalInput")
    kt_d = nc.dram_tensor("kt", [PAIRS, E, S], F32R, kind="ExternalInput")
    vv_d = nc.dram_tensor("vv", [PAIRS, S, E + 1], F32R, kind="ExternalInput")
    deltas_d = nc.dram_tensor("deltas", [128, NT], F32, kind="ExternalInput")
    taub_d = nc.dram_tensor("taub", [128, 1], F32, kind="ExternalInput")
    mask_d = nc.dram_tensor("mask", [128, 128], F32R, kind="ExternalInput")
    eye_d = nc.dram_tensor("eye", [65, 65], F32, kind="ExternalInput")
    out_d = nc.dram_tensor("out", [PAIRS, L, E], F32, kind="ExternalOutput")
    with tile.TileContext(nc) as tc:
        _body(tc, nc, qt_d, kt_d, vv_d, deltas_d, taub_d, mask_d, eye_d, out_d)
    nc.compile()
    _CACHE["nc"] = nc
    return nc


def _in_maps(queries, keys, values, tau, delta):
    qt = np.ascontiguousarray(queries.transpose(0, 2, 3, 1))  # [B, H, E, L]
    kt = np.ascontiguousarray(keys.transpose(0, 2, 3, 1))
    vv = np.concatenate(
        [values.transpose(0, 2, 1, 3), np.ones((B, H, S, 1), np.float32)], axis=3
    )  # [B, H, S, E+1] with ones column for the softmax denominator
    # mask[s, l] = 1 if l >= s else 0 (transposed-layout causal keep-mask)
    mask = (np.arange(128)[None, :] >= np.arange(128)[:, None]).astype(np.float32)
    eye = np.eye(65, dtype=np.float32)
    maps = []
    for i in range(N_CORES):
        b, h0 = i // 2, (i % 2) * PAIRS
        maps.append(
            {
                "qt": qt[b, h0 : h0 + PAIRS],
                "kt": kt[b, h0 : h0 + PAIRS],
                "vv": vv[b, h0 : h0 + PAIRS],
                "deltas": np.ascontiguousarray(
                    delta[b].reshape(NT, 128).T.astype(np.float32)
                ),
                "taub": np.full((128, 1), tau[b, 0], dtype=np.float32),
                "mask": mask,
                "eye": eye,
            }
        )
    return maps


def kernel(queries, keys, values, tau, delta, trace=False, trace_cores=None):
    queries = np.asarray(queries, dtype=np.float32)
    keys = np.asarray(keys, dtype=np.float32)
    values = np.asarray(values, dtype=np.float32)
    tau = np.asarray(tau, dtype=np.float32)
    delta = np.asarray(delta, dtype=np.float32)

    nc = _build()
    maps = _in_maps(queries, keys, values, tau, delta)
    res = bass_utils.run_bass_kernel_spmd(
        nc,
        maps,
        core_ids=list(range(N_CORES)),
        trace=trace,
        trace_cores=trace_cores,
    )
    out = np.empty((B, L, H, E), dtype=np.float32)
    for i in range(N_CORES):
        b, h0 = i // 2, (i % 2) * PAIRS
        o = res.results[i]["out"]  # [PAIRS, L, E]
        for j in range(PAIRS):
            out[b, :, h0 + j, :] = o[j]
    if trace:
        return out, res
    return out


# revision 30
# speedup vs baseline: 6.4654x; 6.4654x over previous
"""Trainium2 Bass kernel for DSAttention (causal attention with per-batch tau
scale and per-key delta bias), B=4 L=S=2048 H=8 E=64 fp32.

Strategy: data+head parallelism across 8 cores (core i -> batch i//2, heads
(i%2)*4 .. +4, i.e. 4 (b,h) pairs per core). Per pair, flash-style attention
computed entirely in the transposed-score layout:

  scoresT[s, l] = sum_e K[s,e] Q[l,e]          (PE, float32r, K/Q pre-transposed
                                                on host so E sits on partitions)
  AT[s, l] = exp((tau/8)*scoresT + delta[s]/8) (ACT; tau via per-partition scale
                                                AP, delta via per-partition bias
                                                AP; no max-subtraction needed --
                                                scores are bounded ~|9|)
  OT[e', l] += V'[s, e'].T @ AT[s, l]          (PE accumulates over s-tiles in
                                                PSUM; V' has a ones column so
                                                row 64 of OT is the softmax
                                                denominator)
  out[l, e] = OT[e, l] / OT[64, l]             (PE transpose of OT chunks, then
                                                DVE reciprocal + scalar mul)

Causality: s-tile si only contributes to columns l >= 128*si; the strict lower
triangle of the diagonal block is zeroed in AT right after the exp (exp inputs
are bounded, so no masking is needed before it).

The emission is software-pipelined over (pair, phase, si, chunk) units: the
next unit's scores matmul is emitted ahead of the previous unit's AV matmul,
OT banks are drained to SBUF the moment their last s-tile lands, and the
normalize/transpose/store tail is spread across later units. PSUM budget:
3 scoresT slots (3 banks) + 2 OT phase accumulators (4 banks) + 1 transpose
bank = 8 banks.
"""

import sys

if "/opt/trn_rl_repo" not in sys.path:
    sys.path.insert(0, "/opt/trn_rl_repo")

import numpy as np

import hashlib

import concourse.bacc as bacc
import concourse.mybir as mybir
import concourse.tile as tile
from concourse import bass2jax as b2j

B, L, S, H, E = 4, 2048, 2048, 8, 64
N_CORES = 8
PAIRS = 4          # (b, h) pairs per core
NT = S // 128      # s-tiles per pair
CHUNK = 1024       # scoresT psum chunk width (2 PSUM banks)
PHW = 1024         # phase width: OT accumulator columns per phase
F32 = mybir.dt.float32
F32R = mybir.dt.float32r
NEG = -1.0e30


def _pieces(lo, hi):
    """Split [lo, hi) at absolute 512 boundaries (PSUM bank alignment)."""
    out = []
    a = lo
    while a < hi:
        b = min(hi, (a // 512 + 1) * 512)
        out.append((a, b))
        a = b
    return out


def _body(tc, nc, qt_d, kt_d, vv_d, deltas_d, taub_d, mask_d, eye_d, out_d):
    Exp = mybir.ActivationFunctionType.Exp
    with (
        tc.tile_pool(name="const", bufs=1) as cp,
        tc.tile_pool(name="io", bufs=2) as iop,
        tc.tile_pool(name="at", bufs=3) as atp,
        tc.tile_pool(name="outp", bufs=2) as op,
        tc.tile_pool(name="ps_s", bufs=2, space="PSUM") as pss,
        tc.tile_pool(name="ps_ot", bufs=1, space="PSUM") as psot,
        tc.tile_pool(name="ps_tr", bufs=2, space="PSUM") as trp,
    ):
        # ---- constants (per core; the core's 4 pairs share one batch b) ----
        delta_raw = cp.tile([128, NT], F32)
        nc.sync.dma_start(delta_raw[:], deltas_d.ap())
        delta_sc = cp.tile([128, NT], F32)
        nc.vector.tensor_scalar_mul(delta_sc[:], delta_raw[:], 0.125)
        tau_raw = cp.tile([128, 1], F32)
        nc.sync.dma_start(tau_raw[:], taub_d.ap())
        tau_sc = cp.tile([128, 1], F32)
        nc.vector.tensor_scalar_mul(tau_sc[:], tau_raw[:], 0.125)
        mask_t = cp.tile([128, 128], F32R)
        nc.sync.dma_start(mask_t[:], mask_d.ap())
        eye_t = cp.tile([65, 65], F32)
        nc.sync.dma_start(eye_t[:], eye_d.ap())

        # ---- software-pipelined emission over all (pair, si, chunk) units ----
        # PE program order interleaves mm1(u+1) ahead of mm2(u) so the PE can
        # compute the next scores block while ACT runs the exp of the current
        # one; per-pair tail work (PSUM drain, transpose, normalize, store) is
        # spread across the following pair's units.
        def load_pair(p):
            qt_t = iop.tile([E, L], F32R, tag="qt")
            kt_t = iop.tile([E, S], F32R, tag="kt")
            vp_t = iop.tile([128, NT * 65], F32R, tag="vp")
            vp3 = vp_t[:].rearrange("p (s c) -> p s c", c=65)
            vsrc = vv_d.ap()[p].rearrange("(s p) e -> p s e", p=128)
            # head pieces first so the first units' matmuls start ASAP
            nc.sync.dma_start(qt_t[:, 0:CHUNK], qt_d.ap()[p][:, 0:CHUNK])
            nc.sync.dma_start(kt_t[:, 0:512], kt_d.ap()[p][:, 0:512])
            nc.sync.dma_start(vp3[:, 0:4, :], vsrc[:, 0:4, :])
            nc.sync.dma_start(qt_t[:, CHUNK:L], qt_d.ap()[p][:, CHUNK:L])
            nc.sync.dma_start(kt_t[:, 512:S], kt_d.ap()[p][:, 512:S])
            nc.sync.dma_start(vp3[:, 4:NT, :], vsrc[:, 4:NT, :])
            # fold tau/8 into Q (avoids the activation scale-as-AP path);
            # piecewise so the first scores matmul only waits on 512 columns
            for z in range(0, L, 512):
                nc.vector.tensor_scalar_mul(
                    qt_t[:, z : z + 512], qt_t[:, z : z + 512], tau_sc[0:E, 0:1]
                )
            return qt_t, kt_t, vp3

        # units: (pair, phase, si, chunk); each phase covers PHW output
        # columns so its OT accumulator is only 2 PSUM banks (double-buffered),
        # which frees a dedicated bank for the tail transposes.
        units = []
        for p in range(PAIRS):
            for ph in range(L // PHW):
                for si in range(NT):
                    for c in range((PHW // CHUNK) * ph, (PHW // CHUNK) * (ph + 1)):
                        if c >= si * 128 // CHUNK:
                            units.append((p, ph, si, c))

        tiles = {0: load_pair(0)}
        ots = {}
        drains = {}  # pair -> (ot_sb, o_all)
        tail = []  # queued closures, a few popped per unit
        pending = []  # (unit, at, lo) with mm2 not yet emitted

        def emit_mm2(u, at, lo):
            p, ph, si, c = u
            _, _, vp3 = tiles[p]
            for a, b in _pieces(lo, (c + 1) * CHUNK):
                bank = a // 512
                nc.tensor.matmul(
                    ots[(p, ph)][:, a - PHW * ph : b - PHW * ph],
                    vp3[:, si, :],
                    at[:, a - lo : b - lo],
                    start=(si == 0),
                    stop=(si == 4 * bank + 3),
                )
            # OT bank `bk` is final once s-tile 4*bk+3 is accumulated; drain
            # it to SBUF immediately and queue its normalize/store tail steps
            # so tail work overlaps the remaining units.
            for a, b in _pieces(lo, (c + 1) * CHUNK):
                bk = a // 512
                if si == 4 * bk + 3:
                    if p not in drains:
                        ot_sb = op.tile([65, L], F32, tag="otsb", name=f"otsb{p}")
                        o_all = op.tile(
                            [128, (L // 128) * 64], F32, tag="oall", name=f"oall{p}"
                        )
                        drains[p] = (ot_sb, o_all)
                    ot_sb = drains[p][0]
                    nc.vector.tensor_copy(
                        ot_sb[:, 512 * bk : 512 * (bk + 1)],
                        ots[(p, ph)][
                            :, 512 * bk - PHW * ph : 512 * (bk + 1) - PHW * ph
                        ],
                    )
                    queue_bank_tail(p, bk)

        def queue_bank_tail(p, bk):
            ot_sb, o_all = drains[p]

            def step(t):
                tr = trp.tile([128, 65], F32, tag="tr")
                nc.tensor.transpose(tr[:], ot_sb[:, 128 * t : 128 * (t + 1)], eye_t[:])
                r = op.tile([128, 1], F32, tag="r")
                nc.vector.reciprocal(r[:], tr[:, 64:65])
                nc.vector.tensor_scalar_mul(
                    o_all[:, 64 * t : 64 * (t + 1)], tr[:, 0:64], r[:, 0:1]
                )
                if t % 4 == 3:  # one store per drained OT bank (4 l-tiles)
                    t0 = t - 3
                    nc.sync.dma_start(
                        out_d.ap()[p, 128 * t0 : 128 * (t + 1), :].rearrange(
                            "(t pp) e -> pp t e", pp=128
                        ),
                        o_all[:, 64 * t0 : 64 * (t + 1)].rearrange(
                            "pp (t e) -> pp t e", e=64
                        ),
                    )

            for t in range(4 * bk, 4 * (bk + 1)):
                tail.append((step, t))

        # index of the unit at which to prefetch the next pair's inputs
        first_unit = {}
        for i, uu in enumerate(units):
            first_unit.setdefault(uu[0], i)
        prefetch_at = {
            first_unit[p + 1] - 8: p + 1 for p in range(PAIRS - 1)
        }

        for i, u in enumerate(units):
            p, ph, si, c = u
            if i in prefetch_at:
                q = prefetch_at[i]
                tiles[q] = load_pair(q)
            if (p, ph) not in ots:
                ots[(p, ph)] = psot.tile(
                    [65, PHW], F32, tag="ot", name=f"ot{p}_{ph}"
                )
            qt_t, kt_t, _ = tiles[p]
            cl = c * CHUNK
            cr = cl + CHUNK
            l0 = si * 128
            lo = max(l0, cl)
            st = pss.tile([128, CHUNK], F32, tag="st")
            for a, b in _pieces(lo, cr):
                nc.tensor.matmul(
                    st[:, a - cl : b - cl],
                    kt_t[:, l0 : l0 + 128],
                    qt_t[:, a:b],
                    start=True,
                    stop=True,
                )
            # tail work (PE transposes + DVE norm) emitted here keeps the DVE
            # FIFO ahead of the post-exp mask below.
            for _ in range(2):
                if tail:
                    fn, t = tail.pop(0)
                    fn(t)
            at = atp.tile([128, CHUNK], F32R, tag="at")
            nc.scalar.activation(
                at[:, 0 : cr - lo],
                st[:, lo - cl : CHUNK],
                Exp,
                bias=delta_sc[:, si : si + 1],
                scale=1.0,
            )
            if cl <= l0 < cr:
                # causal mask: zero the strict lower triangle of the diagonal
                # block AFTER the exp (cheaper than gating the exp on a DVE
                # mask-add; exp inputs are bounded so no overflow risk)
                od = l0 - lo
                nc.vector.tensor_mul(
                    at[:, od : od + 128], at[:, od : od + 128], mask_t[:]
                )
            while len(pending) >= 1:
                q = pending.pop(0)
                emit_mm2(*q)
            pending.append((u, at, lo))

        for q in pending:
            emit_mm2(*q)
        while tail:
            fn, t = tail.pop(0)
            fn(t)


_CACHE = {}


def _build():
    if "nc" in _CACHE:
        return _CACHE["nc"]
    nc = bacc.Bacc("TRN2", target_bir_lowering=False, debug=False, num_devices=N_CORES)
    qt_d = nc.dram_tensor("qt", [PAIRS, E, L], F32R, kind="ExternalInput")
    kt_d = nc.dram_tensor("kt", [PAIRS, E, S], F32R, kind="ExternalInput")
    vv_d = nc.dram_tensor("vv", [PAIRS, S, E + 1], F32R, kind="ExternalInput")
    deltas_d = nc.dram_tensor("deltas", [128, NT], F32, kind="ExternalInput")
    taub_d = nc.dram_tensor("taub", [128, 1], F32, kind="ExternalInput")
    mask_d = nc.dram_tensor("mask", [128, 128], F32R, kind="ExternalInput")
    eye_d = nc.dram_tensor("eye", [65, 65], F32, kind="ExternalInput")
    out_d = nc.dram_tensor("out", [PAIRS, L, E], F32, kind="ExternalOutput")
    with tile.TileContext(nc) as tc:
        _body(tc, nc, qt_d, kt_d, vv_d, deltas_d, taub_d, mask_d, eye_d, out_d)
    nc.compile()
    _CACHE["nc"] = nc
    return nc


def _in_maps(queries, keys, values, tau, delta):
    qt = np.ascontiguousarray(queries.transpose(0, 2, 3, 1))  # [B, H, E, L]
    kt = np.ascontiguousarray(keys.transpose(0, 2, 3, 1))
    vv = np.concatenate(
        [values.transpose(0, 2, 1, 3), np.ones((B, H, S, 1), np.float32)], axis=3
    )  # [B, H, S, E+1] with ones column for the softmax denominator
    # mask[s, l] = 1 if l >= s else 0 (transposed-layout causal keep-mask)
    mask = (np.arange(128)[None, :] >= np.arange(128)[:, None]).astype(np.float32)
    eye = np.eye(65, dtype=np.float32)
    maps = []
    for i in range(N_CORES):
        b, h0 = i // 2, (i % 2) * PAIRS
        maps.append(
            {
                "qt": qt[b, h0 : h0 + PAIRS],
                "kt": kt[b, h0 : h0 + PAIRS],
                "vv": vv[b, h0 : h0 + PAIRS],
                "deltas": np.ascontiguousarray(
                    delta[b].reshape(NT, 128).T.astype(np.float32)
                ),
                "taub": np.full((128, 1), tau[b, 0], dtype=np.float32),
                "mask": mask,
                "eye": eye,
            }
        )
    return maps


def _make_runner(nc):
    """Jitted 8-core SPMD runner for the prebuilt Bass module (mirrors
    bass2jax.run_bass_via_pjrt, but reusable across calls and with the traced
    function named by BIR hash -- the neuron compile cache keys on the HLO
    module name + shapes and NOT on the embedded BIR payload, so a content
    hash in the name prevents stale-NEFF cache hits)."""
    import jax
    from jax.experimental.shard_map import shard_map
    from jax.sharding import Mesh, NamedSharding, PartitionSpec

    b2j.install_neuronx_cc_hook()
    partition_name = nc.partition_id_tensor.name if nc.partition_id_tensor else None
    in_names, out_names, out_avals, zero_outs = [], [], [], []
    for alloc in nc.m.functions[0].allocations:
        if not isinstance(alloc, mybir.MemoryLocationSet):
            continue
        name = alloc.memorylocations[0].name
        if alloc.kind == "ExternalInput":
            if name != partition_name:
                in_names.append(name)
        elif alloc.kind == "ExternalOutput":
            out_names.append(name)
            shape = tuple(alloc.tensor_shape)
            dtype = mybir.dt.np(alloc.dtype)
            out_avals.append(jax.core.ShapedArray(shape, dtype))
            zero_outs.append(np.zeros(shape, dtype))
    n_params = len(in_names)
    n_outs = len(out_avals)
    all_in_names = list(in_names) + list(out_names)
    if partition_name is not None:
        all_in_names.append(partition_name)

    def _body(*args):
        operands = list(args)
        if partition_name is not None:
            operands.append(b2j.partition_id_tensor())
        outs = b2j._bass_exec_p.bind(
            *operands,
            out_avals=tuple(out_avals),
            in_names=tuple(all_in_names),
            out_names=tuple(out_names),
            lowering_input_output_aliases=(),
            sim_require_finite=True,
            sim_require_nnan=True,
            nc=nc,
        )
        return tuple(outs)

    bir_hash = hashlib.sha256(nc.to_json_bytes()).hexdigest()[:12]
    _body.__name__ = f"_body_{bir_hash}"

    devices = jax.devices()[:N_CORES]
    assert len(devices) == N_CORES, f"need {N_CORES} devices, have {len(devices)}"
    mesh = Mesh(np.asarray(devices), ("core",))
    in_specs = (PartitionSpec("core"),) * (n_params + n_outs)
    out_specs = (PartitionSpec("core"),) * n_outs
    donate = tuple(range(n_params, n_params + n_outs))
    sharded = jax.jit(
        shard_map(
            _body, mesh=mesh, in_specs=in_specs, out_specs=out_specs, check_rep=False
        ),
        donate_argnums=donate,
        keep_unused=True,
    )
    sh = NamedSharding(mesh, PartitionSpec("core"))

    def run(maps):
        concat_in = [
            np.concatenate([np.asarray(maps[c][nm]) for c in range(N_CORES)], axis=0)
            for nm in in_names
        ]
        dev_in = [jax.device_put(x, sh) for x in concat_in]
        zeros = [
            jax.device_put(
                np.zeros((N_CORES * z.shape[0], *z.shape[1:]), z.dtype), sh
            )
            for z in zero_outs
        ]
        outs = sharded(*dev_in, *zeros)
        jax.block_until_ready(outs)
        return {
            name: np.asarray(outs[i]).reshape(N_CORES, *out_avals[i].shape)
            for i, name in enumerate(out_names)
        }

    return run


def kernel(queries, keys, values, tau, delta):
    queries = np.asarray(queries, dtype=np.float32)
    keys = np.asarray(keys, dtype=np.float32)
    values = np.asarray(values, dtype=np.float32)
    tau = np.asarray(tau, dtype=np.float32)
    delta = np.asarray(delta, dtype=np.float32)

    nc = _build()
    if "runner" not in _CACHE:
        _CACHE["runner"] = _make_runner(nc)
    maps = _in_maps(queries, keys, values, tau, delta)
    res = _CACHE["runner"](maps)
    out = np.empty((B, L, H, E), dtype=np.float32)
    for i in range(N_CORES):
        b, h0 = i // 2, (i % 2) * PAIRS
        o = res["out"][i]  # [PAIRS, L, E]
        for j in range(PAIRS):
            out[b, :, h0 + j, :] = o[j]
    return out


# revision 32
# speedup vs baseline: 10.8057x; 1.6713x over previous
"""Trainium2 Bass kernel for DSAttention (causal attention with per-batch tau
scale and per-key delta bias), B=4 L=S=2048 H=8 E=64 fp32.

Strategy: data+head parallelism across 8 cores (core i -> batch i//2, heads
(i%2)*4 .. +4, i.e. 4 (b,h) pairs per core). Per pair, flash-style attention
computed entirely in the transposed-score layout:

  scoresT[s, l] = sum_e K[s,e] Q[l,e]          (PE, float32r, K/Q pre-transposed
                                                on host so E sits on partitions)
  AT[s, l] = exp((tau/8)*scoresT + delta[s]/8) (ACT; tau via per-partition scale
                                                AP, delta via per-partition bias
                                                AP; no max-subtraction needed --
                                                scores are bounded ~|9|)
  OT[e', l] += V'[s, e'].T @ AT[s, l]          (PE accumulates over s-tiles in
                                                PSUM; V' has a ones column so
                                                row 64 of OT is the softmax
                                                denominator)
  out[l, e] = OT[e, l] / OT[64, l]             (PE transpose of OT chunks, then
                                                DVE reciprocal + scalar mul)

Causality: s-tile si only contributes to columns l >= 128*si; the strict lower
triangle of the diagonal block is zeroed in AT right after the exp (exp inputs
are bounded, so no masking is needed before it).

The emission is software-pipelined over (pair, phase, si, chunk) units: the
next unit's scores matmul is emitted ahead of the previous unit's AV matmul,
OT banks are drained to SBUF the moment their last s-tile lands, and the
normalize/transpose/store tail is spread across later units. PSUM budget:
3 scoresT slots (3 banks) + 2 OT phase accumulators (4 banks) + 1 transpose
bank = 8 banks.
"""

import sys

if "/opt/trn_rl_repo" not in sys.path:
    sys.path.insert(0, "/opt/trn_rl_repo")

import numpy as np

import hashlib

import concourse.bacc as bacc
import concourse.mybir as mybir
import concourse.tile as tile
from concourse import bass2jax as b2j

B, L, S, H, E = 4, 2048, 2048, 8, 64
N_CORES = 8
PAIRS = 4          # (b, h) pairs per core
NT = S // 128      # s-tiles per pair
CHUNK = 1024       # scoresT psum chunk width (2 PSUM banks)
PHW = 1024         # phase width: OT accumulator columns per phase
F32 = mybir.dt.float32
F32R = mybir.dt.float32r
NEG = -1.0e30


def _pieces(lo, hi):
    """Split [lo, hi) at absolute 512 boundaries (PSUM bank alignment)."""
    out = []
    a = lo
    while a < hi:
        b = min(hi, (a // 512 + 1) * 512)
        out.append((a, b))
        a = b
    return out


def _body(tc, nc, qt_d, kt_d, vv_d, deltas_d, taub_d, mask_d, eye_d, out_d):
    Exp = mybir.ActivationFunctionType.Exp
    with (
        tc.tile_pool(name="const", bufs=1) as cp,
        tc.tile_pool(name="io", bufs=2) as iop,
        tc.tile_pool(name="at", bufs=3) as atp,
        tc.tile_pool(name="outp", bufs=2) as op,
        tc.tile_pool(name="ps_s", bufs=2, space="PSUM") as pss,
        tc.tile_pool(name="ps_ot", bufs=1, space="PSUM") as psot,
        tc.tile_pool(name="ps_tr", bufs=2, space="PSUM") as trp,
    ):
        # ---- constants; only tau must precede the first input loads (the
        # qt scale reads it), the rest are interleaved into pair 0's load
        # sequence so they don't serialize the startup DMA triggers ----
        tau_raw = cp.tile([128, 1], F32)
        nc.sync.dma_start(tau_raw[:], taub_d.ap())
        tau_sc = cp.tile([128, 1], F32)
        nc.vector.tensor_scalar_mul(tau_sc[:], tau_raw[:], 0.125)
        delta_sc = cp.tile([128, NT], F32)
        mask_t = cp.tile([128, 128], F32R)
        eye_t = cp.tile([65, 65], F32)

        def load_mid_consts():
            delta_raw = cp.tile([128, NT], F32)
            nc.sync.dma_start(delta_raw[:], deltas_d.ap())
            nc.vector.tensor_scalar_mul(delta_sc[:], delta_raw[:], 0.125)
            nc.sync.dma_start(mask_t[:], mask_d.ap())

        def load_late_consts():
            nc.sync.dma_start(eye_t[:], eye_d.ap())

        # ---- software-pipelined emission over all (pair, si, chunk) units ----
        # PE program order interleaves mm1(u+1) ahead of mm2(u) so the PE can
        # compute the next scores block while ACT runs the exp of the current
        # one; per-pair tail work (PSUM drain, transpose, normalize, store) is
        # spread across the following pair's units.
        def load_pair(p):
            qt_t = iop.tile([E, L], F32R, tag="qt")
            kt_t = iop.tile([E, S], F32R, tag="kt")
            vp_t = iop.tile([128, NT * 65], F32R, tag="vp")
            vp3 = vp_t[:].rearrange("p (s c) -> p s c", c=65)
            vsrc = vv_d.ap()[p].rearrange("(s p) e -> p s e", p=128)
            # interleaved pieces ordered by first use so early units never
            # wait behind a bulk transfer
            nc.sync.dma_start(qt_t[:, 0:CHUNK], qt_d.ap()[p][:, 0:CHUNK])
            nc.sync.dma_start(kt_t[:, 0:512], kt_d.ap()[p][:, 0:512])
            if p == 0:
                load_mid_consts()
            nc.sync.dma_start(vp3[:, 0:4, :], vsrc[:, 0:4, :])
            nc.sync.dma_start(kt_t[:, 512:1024], kt_d.ap()[p][:, 512:1024])
            nc.sync.dma_start(vp3[:, 4:8, :], vsrc[:, 4:8, :])
            nc.sync.dma_start(qt_t[:, CHUNK:L], qt_d.ap()[p][:, CHUNK:L])
            nc.sync.dma_start(kt_t[:, 1024:S], kt_d.ap()[p][:, 1024:S])
            nc.sync.dma_start(vp3[:, 8:NT, :], vsrc[:, 8:NT, :])
            if p == 0:
                load_late_consts()
            # fold tau/8 into Q (avoids the activation scale-as-AP path);
            # piecewise so the first scores matmul only waits on 512 columns
            for z in range(0, L, 512):
                nc.vector.tensor_scalar_mul(
                    qt_t[:, z : z + 512], qt_t[:, z : z + 512], tau_sc[0:E, 0:1]
                )
            return qt_t, kt_t, vp3

        # units: (pair, phase, si, chunk); each phase covers PHW output
        # columns so its OT accumulator is only 2 PSUM banks (double-buffered),
        # which frees a dedicated bank for the tail transposes.
        units = []
        for p in range(PAIRS):
            for ph in range(L // PHW):
                for si in range(NT):
                    for c in range((PHW // CHUNK) * ph, (PHW // CHUNK) * (ph + 1)):
                        if c >= si * 128 // CHUNK:
                            units.append((p, ph, si, c))

        tiles = {0: load_pair(0)}
        ots = {}
        drains = {}  # pair -> (ot_sb, o_all)
        tail = []  # queued closures, a few popped per unit
        pending = []  # (unit, at, lo) with mm2 not yet emitted

        def emit_mm2(u, at, lo):
            p, ph, si, c = u
            _, _, vp3 = tiles[p]
            for a, b in _pieces(lo, (c + 1) * CHUNK):
                bank = a // 512
                nc.tensor.matmul(
                    ots[(p, ph)][:, a - PHW * ph : b - PHW * ph],
                    vp3[:, si, :],
                    at[:, a - lo : b - lo],
                    start=(si == 0),
                    stop=(si == 4 * bank + 3),
                )
            # OT bank `bk` is final once s-tile 4*bk+3 is accumulated; drain
            # it to SBUF immediately and queue its normalize/store tail steps
            # so tail work overlaps the remaining units.
            for a, b in _pieces(lo, (c + 1) * CHUNK):
                bk = a // 512
                if si == 4 * bk + 3:
                    if p not in drains:
                        ot_sb = op.tile([65, L], F32, tag="otsb", name=f"otsb{p}")
                        o_all = op.tile(
                            [128, (L // 128) * 64], F32, tag="oall", name=f"oall{p}"
                        )
                        drains[p] = (ot_sb, o_all)
                    ot_sb = drains[p][0]
                    nc.vector.tensor_copy(
                        ot_sb[:, 512 * bk : 512 * (bk + 1)],
                        ots[(p, ph)][
                            :, 512 * bk - PHW * ph : 512 * (bk + 1) - PHW * ph
                        ],
                    )
                    queue_bank_tail(p, bk)

        def queue_bank_tail(p, bk):
            ot_sb, o_all = drains[p]

            def step(t):
                tr = trp.tile([128, 65], F32, tag="tr")
                nc.tensor.transpose(tr[:], ot_sb[:, 128 * t : 128 * (t + 1)], eye_t[:])
                r = op.tile([128, 1], F32, tag="r")
                nc.vector.reciprocal(r[:], tr[:, 64:65])
                nc.vector.tensor_scalar_mul(
                    o_all[:, 64 * t : 64 * (t + 1)], tr[:, 0:64], r[:, 0:1]
                )
                if t % 4 == 3:  # one store per drained OT bank (4 l-tiles)
                    t0 = t - 3
                    nc.sync.dma_start(
                        out_d.ap()[p, 128 * t0 : 128 * (t + 1), :].rearrange(
                            "(t pp) e -> pp t e", pp=128
                        ),
                        o_all[:, 64 * t0 : 64 * (t + 1)].rearrange(
                            "pp (t e) -> pp t e", e=64
                        ),
                    )

            for t in range(4 * bk, 4 * (bk + 1)):
                tail.append((step, t))

        # index of the unit at which to prefetch the next pair's inputs
        first_unit = {}
        for i, uu in enumerate(units):
            first_unit.setdefault(uu[0], i)
        prefetch_at = {
            first_unit[p + 1] - 12: p + 1 for p in range(PAIRS - 1)
        }

        for i, u in enumerate(units):
            p, ph, si, c = u
            if i in prefetch_at:
                q = prefetch_at[i]
                tiles[q] = load_pair(q)
            if (p, ph) not in ots:
                ots[(p, ph)] = psot.tile(
                    [65, PHW], F32, tag="ot", name=f"ot{p}_{ph}"
                )
            qt_t, kt_t, _ = tiles[p]
            cl = c * CHUNK
            cr = cl + CHUNK
            l0 = si * 128
            lo = max(l0, cl)
            st = pss.tile([128, CHUNK], F32, tag="st")
            for a, b in _pieces(lo, cr):
                nc.tensor.matmul(
                    st[:, a - cl : b - cl],
                    kt_t[:, l0 : l0 + 128],
                    qt_t[:, a:b],
                    start=True,
                    stop=True,
                )
            # tail work (PE transposes + DVE norm) emitted here keeps the DVE
            # FIFO ahead of the post-exp mask below.
            for _ in range(2):
                if tail:
                    fn, t = tail.pop(0)
                    fn(t)
            at = atp.tile([128, CHUNK], F32R, tag="at")
            nc.scalar.activation(
                at[:, 0 : cr - lo],
                st[:, lo - cl : CHUNK],
                Exp,
                bias=delta_sc[:, si : si + 1],
                scale=1.0,
            )
            if cl <= l0 < cr:
                # causal mask: zero the strict lower triangle of the diagonal
                # block AFTER the exp (cheaper than gating the exp on a DVE
                # mask-add; exp inputs are bounded so no overflow risk)
                od = l0 - lo
                nc.vector.tensor_mul(
                    at[:, od : od + 128], at[:, od : od + 128], mask_t[:]
                )
            while len(pending) >= 1:
                q = pending.pop(0)
                emit_mm2(*q)
            pending.append((u, at, lo))

        for q in pending:
            emit_mm2(*q)
        while tail:
            fn, t = tail.pop(0)
            fn(t)


_CACHE = {}


def _build():
    if "nc" in _CACHE:
        return _CACHE["nc"]
    nc = bacc.Bacc("TRN2", target_bir_lowering=False, debug=False, num_devices=N_CORES)
    qt_d = nc.dram_tensor("qt", [PAIRS, E, L], F32R, kind="ExternalInput")
    kt_d = nc.dram_tensor("kt", [PAIRS, E, S], F32R, kind="ExternalInput")
    vv_d = nc.dram_tensor("vv", [PAIRS, S, E + 1], F32R, kind="ExternalInput")
    deltas_d = nc.dram_tensor("deltas", [128, NT], F32, kind="ExternalInput")
    taub_d = nc.dram_tensor("taub", [128, 1], F32, kind="ExternalInput")
    mask_d = nc.dram_tensor("mask", [128, 128], F32R, kind="ExternalInput")
    eye_d = nc.dram_tensor("eye", [65, 65], F32, kind="ExternalInput")
    out_d = nc.dram_tensor("out", [PAIRS, L, E], F32, kind="ExternalOutput")
    with tile.TileContext(nc) as tc:
        _body(tc, nc, qt_d, kt_d, vv_d, deltas_d, taub_d, mask_d, eye_d, out_d)
    nc.compile()
    _CACHE["nc"] = nc
    return nc


def _in_maps(queries, keys, values, tau, delta):
    qt = np.ascontiguousarray(queries.transpose(0, 2, 3, 1))  # [B, H, E, L]
    kt = np.ascontiguousarray(keys.transpose(0, 2, 3, 1))
    vv = np.concatenate(
        [values.transpose(0, 2, 1, 3), np.ones((B, H, S, 1), np.float32)], axis=3
    )  # [B, H, S, E+1] with ones column for the softmax denominator
    # mask[s, l] = 1 if l >= s else 0 (transposed-layout causal keep-mask)
    mask = (np.arange(128)[None, :] >= np.arange(128)[:, None]).astype(np.float32)
    eye = np.eye(65, dtype=np.float32)
    maps = []
    for i in range(N_CORES):
        b, h0 = i // 2, (i % 2) * PAIRS
        maps.append(
            {
                "qt": qt[b, h0 : h0 + PAIRS],
                "kt": kt[b, h0 : h0 + PAIRS],
                "vv": vv[b, h0 : h0 + PAIRS],
                "deltas": np.ascontiguousarray(
                    delta[b].reshape(NT, 128).T.astype(np.float32)
                ),
                "taub": np.full((128, 1), tau[b, 0], dtype=np.float32),
                "mask": mask,
                "eye": eye,
            }
        )
    return maps


def _make_runner(nc):
    """Jitted 8-core SPMD runner for the prebuilt Bass module (mirrors
    bass2jax.run_bass_via_pjrt, but reusable across calls and with the traced
    function named by BIR hash -- the neuron compile cache keys on the HLO
    module name + shapes and NOT on the embedded BIR payload, so a content
    hash in the name prevents stale-NEFF cache hits)."""
    import jax
    from jax.experimental.shard_map import shard_map
    from jax.sharding import Mesh, NamedSharding, PartitionSpec

    b2j.install_neuronx_cc_hook()
    partition_name = nc.partition_id_tensor.name if nc.partition_id_tensor else None
    in_names, out_names, out_avals, zero_outs = [], [], [], []
    for alloc in nc.m.functions[0].allocations:
        if not isinstance(alloc, mybir.MemoryLocationSet):
            continue
        name = alloc.memorylocations[0].name
        if alloc.kind == "ExternalInput":
            if name != partition_name:
                in_names.append(name)
        elif alloc.kind == "ExternalOutput":
            out_names.append(name)
            shape = tuple(alloc.tensor_shape)
            dtype = mybir.dt.np(alloc.dtype)
            out_avals.append(jax.core.ShapedArray(shape, dtype))
            zero_outs.append(np.zeros(shape, dtype))
    n_params = len(in_names)
    n_outs = len(out_avals)
    all_in_names = list(in_names) + list(out_names)
    if partition_name is not None:
        all_in_names.append(partition_name)

    def _body(*args):
        operands = list(args)
        if partition_name is not None:
            operands.append(b2j.partition_id_tensor())
        outs = b2j._bass_exec_p.bind(
            *operands,
            out_avals=tuple(out_avals),
            in_names=tuple(all_in_names),
            out_names=tuple(out_names),
            lowering_input_output_aliases=(),
            sim_require_finite=True,
            sim_require_nnan=True,
            nc=nc,
        )
        return tuple(outs)

    bir_hash = hashlib.sha256(nc.to_json_bytes()).hexdigest()[:12]
    _body.__name__ = f"_body_{bir_hash}"

    devices = jax.devices()[:N_CORES]
    assert len(devices) == N_CORES, f"need {N_CORES} devices, have {len(devices)}"
    mesh = Mesh(np.asarray(devices), ("core",))
    in_specs = (PartitionSpec("core"),) * (n_params + n_outs)
    out_specs = (PartitionSpec("core"),) * n_outs
    donate = tuple(range(n_params, n_params + n_outs))
    sharded = jax.jit(
        shard_map(
            _body, mesh=mesh, in_specs=in_specs, out_specs=out_specs, check_rep=False
        ),
        donate_argnums=donate,
        keep_unused=True,
    )
    sh = NamedSharding(mesh, PartitionSpec("core"))

    def run(maps):
        concat_in = [
            np.concatenate([np.asarray(maps[c][nm]) for c in range(N_CORES)], axis=0)
            for nm in in_names
        ]
        dev_in = [jax.device_put(x, sh) for x in concat_in]
        zeros = [
            jax.device_put(
                np.zeros((N_CORES * z.shape[0], *z.shape[1:]), z.dtype), sh
            )
            for z in zero_outs
        ]
        outs = sharded(*dev_in, *zeros)
        jax.block_until_ready(outs)
        return {
            name: np.asarray(outs[i]).reshape(N_CORES, *out_avals[i].shape)
            for i, name in enumerate(out_names)
        }

    return run


def kernel(queries, keys, values, tau, delta):
    queries = np.asarray(queries, dtype=np.float32)
    keys = np.asarray(keys, dtype=np.float32)
    values = np.asarray(values, dtype=np.float32)
    tau = np.asarray(tau, dtype=np.float32)
    delta = np.asarray(delta, dtype=np.float32)

    nc = _build()
    if "runner" not in _CACHE:
        _CACHE["runner"] = _make_runner(nc)
    maps = _in_maps(queries, keys, values, tau, delta)
    res = _CACHE["runner"](maps)
    out = np.empty((B, L, H, E), dtype=np.float32)
    for i in range(N_CORES):
        b, h0 = i // 2, (i % 2) * PAIRS
        o = res["out"][i]  # [PAIRS, L, E]
        for j in range(PAIRS):
            out[b, :, h0 + j, :] = o[j]
    return out
